# revision 1
# baseline (speedup 1.0000x reference)
"""Chamfer loss kernel for 8 Trainium2 NeuronCores.

Problem: f, f_ of shape [8, 4096, 3] fp32; loss = mean_b [ mean_n min_m D + mean_m min_n D ]
where D is the [4096, 4096] squared-distance matrix per batch.

Sharding: batch-parallel, one batch per core (8 cores).

Per-core algorithm:
  D[n,m] = ||f_n||^2 + ||g_m||^2 - 2 f_n.g_m
  min over m:  ||f_n||^2 + min_m(-2 f.g + ||g_m||^2)   -> dir-1 matmul, row-min
  min over n:  ||g_m||^2 + min_n(-2 g.f + ||f_n||^2)   -> dir-2 matmul (roles swapped), row-min
The -2x.y + ||y||^2 term is computed as ONE augmented matmul on the tensor
engine with a bf16 hi/lo split of the inputs (error ~1e-6 before the final
bf16 rounding of the PSUM output, ~0.2% relative on each distance, which
averages out to ~1e-4 on the mean-of-mins).

Row-mins are computed with tensor_reduce(min) ops on the vector engine
reading [128, 2048] fp32 spans straight from PSUM (the DVE is the only
engine that can min-reduce; measured ~0.95 cycles/element, ~97% busy).
Per-block mins accumulate in SBUF; a small epilogue pairs the per-tile
mins, sums over blocks, and DMAs per-partition sums [128, 2] out; the host
adds the norm means and averages over batches.

Blocks are scheduled in a mixed pattern (default ABBABB): 'A' blocks are
min-reduced by the DVE straight from PSUM; 'B' blocks are drained by the
otherwise-idle ScalarE (PSUM -> SBUF bf16 copies) and min-combined by the
DVE with bf16 tensor_tensor min ops (2x/4x perf modes) plus one small
reduce, with the TT tree emitted one block late so the DVE FIFO never
head-blocks on ScalarE. The mix balances PSUM egress across both engines
(~1 elem/lane/cycle each). Measured on HW: A-only 299us, 2/3-B 269us.

Alternatives measured and rejected on HW: tensor_tensor_reduce (hangs the
exec unit on this runtime), gpsimd tensor_tensor (fails walrus codegen),
and DMA from PSUM (not supported).
"""

import os
import sys

import numpy as np

for _p in ("/opt/trn_rl_repo",):
    if _p not in sys.path and os.path.isdir(_p):
        sys.path.append(_p)

import ml_dtypes  # noqa: E402

BF16 = ml_dtypes.bfloat16

B, N, M, C = 8, 4096, 4096, 3
NBLK = 128          # rows per n-block (PSUM partition dim)
MTILE = 2048        # columns per PSUM tile (fp32 -> 4 banks)
MMN = 512           # matmul free dim (one PSUM bank of fp32)
K = 15              # augmented contraction dim (rows 0..14), row 15 zero pad
KP = 16             # padded partition count of the input arrays


# ----------------------------------------------------------------- host prep
def _bf16_split(x):
    """x (f32/f64) -> (hi, lo) bf16 arrays with hi+lo ~ x (16-bit mantissa)."""
    hi = x.astype(BF16)
    lo = (x.astype(np.float64) - hi.astype(np.float64)).astype(BF16)
    return hi, lo


def _prep_batch(f, g):
    """Build the 4 augmented [KP, 4096] bf16 operand arrays for one batch.

    W(x): stationary form of y = -2x : rows [yh,yh,yl,yl (3 each), 1,1,1, 0]
    S(x): moving form of x          : rows [xh,xl,xh,xl (3 each), n1,n2,n3, 0]
    so that W(a).T @ S(b) = -2 a.b + ||b||^2  (exact products, 3-way split norm).
    """
    def w_form(x):
        y = -2.0 * x.astype(np.float64)  # [n, 3]
        yh, yl = _bf16_split(y)
        out = np.zeros((KP, x.shape[0]), dtype=BF16)
        out[0:3] = yh.T
        out[3:6] = yh.T
        out[6:9] = yl.T
        out[9:12] = yl.T
        out[12:15] = np.ones((3, x.shape[0]), dtype=BF16)
        return out

    def s_form(x):
        xd = x.astype(np.float64)
        xh, xl = _bf16_split(xd)
        nrm = (xd * xd).sum(axis=1)  # [n]
        n1 = nrm.astype(BF16)
        n2 = (nrm - n1.astype(np.float64)).astype(BF16)
        n3 = (nrm - n1.astype(np.float64) - n2.astype(np.float64)).astype(BF16)
        out = np.zeros((KP, x.shape[0]), dtype=BF16)
        out[0:3] = xh.T
        out[3:6] = xl.T
        out[6:9] = xh.T
        out[9:12] = xl.T
        out[12] = n1
        out[13] = n2
        out[14] = n3
        return out

    return {
        "wf": np.ascontiguousarray(w_form(f)),
        "sg": np.ascontiguousarray(s_form(g)),
        "wg": np.ascontiguousarray(w_form(g)),
        "sf": np.ascontiguousarray(s_form(f)),
    }


# ------------------------------------------------------------- device program
def build_program(num_devices, n_points=N, m_points=M, repeat=1, hw_repeat=1,
                  pattern="ABBABB"):
    """Build the Bass program. Returns (nc, names).

    n_points: number of f-points (rows) -- must be multiple of 128.
    m_points: number of g-points (cols) -- must be multiple of MTILE.
    repeat: python-unrolled repetitions (for timing).
    hw_repeat: hardware For_i loop repetitions around the body (for timing).
    pattern: per-block schedule. 'A' = DVE reduces both PSUM tiles directly;
      'B' = ScalarE copies both PSUM tiles to SBUF bf16, DVE does a 4x-mode
      tensor_tensor min tree + small reduce. Mixing balances PSUM egress
      between the two engines (~1 elem/lane/cycle each).
    """
    import concourse.bass as bass
    import concourse.mybir as mybir
    from concourse import bacc, tile

    f32 = mybir.dt.float32
    bf16 = mybir.dt.bfloat16
    AL = mybir.AluOpType

    nb1 = n_points // NBLK          # dir-1 n-blocks
    nt1 = m_points // MTILE         # dir-1 psum tiles per block
    nb2 = m_points // NBLK          # dir-2 blocks (roles swapped)
    nt2 = n_points // MTILE

    nc = bacc.Bacc("TRN2", target_bir_lowering=False, debug=False,
                   num_devices=num_devices)

    wf = nc.dram_tensor("wf", [KP, n_points], bf16, kind="ExternalInput")
    sg = nc.dram_tensor("sg", [KP, m_points], bf16, kind="ExternalInput")
    wg = nc.dram_tensor("wg", [KP, m_points], bf16, kind="ExternalInput")
    sf = nc.dram_tensor("sf", [KP, n_points], bf16, kind="ExternalInput")
    out = nc.dram_tensor("out", [128, 2], f32, kind="ExternalOutput")

    with tile.TileContext(nc) as tc:
        with (
            tc.tile_pool(name="inp", bufs=1) as inp,
            tc.tile_pool(name="psum", bufs=2, space="PSUM") as psum,
            tc.tile_pool(name="scratch", bufs=4) as scratch,
            tc.tile_pool(name="minb", bufs=2) as minb,
            tc.tile_pool(name="outp", bufs=2) as outp,
        ):
            wf_t = inp.tile([KP, n_points], bf16, tag="wf")
            sg_t = inp.tile([KP, m_points], bf16, tag="sg")
            wg_t = inp.tile([KP, m_points], bf16, tag="wg")
            sf_t = inp.tile([KP, n_points], bf16, tag="sf")
            nc.sync.dma_start(wf_t[:], wf.ap())
            nc.sync.dma_start(sg_t[:], sg.ap())
            nc.sync.dma_start(wg_t[:], wg.ap())
            nc.sync.dma_start(sf_t[:], sf.ap())

            def body(_iv=None):
                out_t = outp.tile([128, 2], f32, tag="out")
                for d, (w_t, s_t, nb, nt) in enumerate((
                    (wf_t, sg_t, nb1, nt1),
                    (wg_t, sf_t, nb2, nt2),
                )):
                    # block types: 'A' = DVE reduces each PSUM tile
                    # directly (cols in groups of nt, min-combined in the
                    # epilogue); 'B' (nt==4 only) = ScalarE copies all four
                    # PSUM tiles to SBUF bf16, DVE runs a 4x-mode TT-min tree
                    if nt == 2:
                        plan = (pattern * nb)[:nb]
                    else:
                        plan = "A" * nb
                    na = plan.count("A")
                    nbb = nb - na
                    minbuf = minb.tile([128, nt * na + nbb], f32, tag="minbuf")
                    acol = 0
                    bcol = nt * na

                    def make_tree(cps, col):
                        # emitted one block late so DVE's FIFO queue never
                        # head-blocks on ScalarE copies still in flight
                        def emit():
                            # every TT reads two DIFFERENT tiles: measured
                            # ~4x mode; same-tile halves only reach ~2x
                            h2 = MTILE // 2
                            ga = scratch.tile([128, h2], bf16, tag="ga")
                            nc.vector.tensor_tensor(
                                out=ga[:], in0=cps[0][:, 0:h2],
                                in1=cps[1][:, 0:h2], op=AL.min)
                            gb = scratch.tile([128, h2], bf16, tag="gb")
                            nc.vector.tensor_tensor(
                                out=gb[:], in0=cps[0][:, h2:MTILE],
                                in1=cps[1][:, h2:MTILE], op=AL.min)
                            h_ = scratch.tile([128, h2], bf16, tag="h")
                            nc.vector.tensor_tensor(
                                out=h_[:], in0=ga[:], in1=gb[:], op=AL.min)
                            h4 = h2 // 2
                            j_ = scratch.tile([128, h4], bf16, tag="j")
                            nc.vector.tensor_tensor(
                                out=j_[:], in0=h_[:, 0:h4], in1=h_[:, h4:h2],
                                op=AL.min)
                            nc.vector.tensor_reduce(
                                out=minbuf[:, col:col + 1],
                                in_=j_[:],
                                axis=mybir.AxisListType.X,
                                op=AL.min,
                            )
                        return emit

                    pending = []
                    for i in range(nb):
                        typ = plan[i]
                        lhsT = w_t[0:K, NBLK * i:NBLK * (i + 1)]
                        cps = []
                        for t in range(nt):
                            pt = psum.tile([128, MTILE], f32, tag="ps")
                            for h in range(MTILE // MMN):
                                m0 = MTILE * t + MMN * h
                                nc.tensor.matmul(
                                    pt[:, MMN * h:MMN * (h + 1)],
                                    lhsT,
                                    s_t[0:K, m0:m0 + MMN],
                                    start=True, stop=True,
                                )
                            if typ == "A":
                                nc.vector.tensor_reduce(
                                    out=minbuf[:, acol:acol + 1],
                                    in_=pt[:],
                                    axis=mybir.AxisListType.X,
                                    op=AL.min,
                                )
                                acol += 1
                            else:
                                cp = scratch.tile([128, MTILE], bf16,
                                                  tag=f"cp{t}")
                                nc.scalar.copy(cp[:], pt[:])
                                cps.append(cp)
                        if len(pending) > 1:
                            pending.pop(0)()
                        if typ == "B":
                            pending.append(make_tree(cps, bcol))
                            bcol += 1
                    for fn in pending:
                        fn()
                    # epilogue: out[:, d] = sum(min over A tile-groups) + sum(B)
                    parts = []
                    if na:
                        if nt == 1:
                            amins = minbuf[:, 0:na]
                        else:
                            mb = minbuf[:, 0:nt * na].rearrange(
                                "p (i q) -> p i q", q=nt)
                            sc2 = scratch.tile([128, na], f32, tag="sc2")
                            nc.vector.tensor_reduce(
                                out=sc2[:], in_=mb, axis=mybir.AxisListType.X,
                                op=AL.min)
                            amins = sc2[:]
                        pa = scratch.tile([128, 1], f32, tag="pa")
                        nc.vector.tensor_reduce(
                            out=pa[:], in_=amins,
                            axis=mybir.AxisListType.X, op=AL.add)
                        parts.append(pa)
                    if nbb:
                        pb = scratch.tile([128, 1], f32, tag="pb")
                        nc.vector.tensor_reduce(
                            out=pb[:], in_=minbuf[:, nt * na:nt * na + nbb],
                            axis=mybir.AxisListType.X, op=AL.add)
                        parts.append(pb)
                    if len(parts) == 2:
                        nc.vector.tensor_tensor(
                            out=out_t[:, d:d + 1], in0=parts[0][:],
                            in1=parts[1][:], op=AL.add)
                    else:
                        nc.vector.tensor_copy(out_t[:, d:d + 1], parts[0][:])
                nc.sync.dma_start(out.ap(), out_t[:])

            if hw_repeat > 1:
                with tc.For_i(0, hw_repeat, 1) as iv:
                    for _ in range(repeat):
                        body(iv)
            else:
                for _ in range(repeat):
                    body()

    nc.compile()
    return nc


# ----------------------------------------------------------------- entrypoint
_CACHE = {}


def _get_program(num_devices=8, repeat=1, hw_repeat=1, pattern="ABBABB"):
    key = (num_devices, repeat, hw_repeat, pattern)
    if key not in _CACHE:
        _CACHE[key] = build_program(num_devices, repeat=repeat,
                                    hw_repeat=hw_repeat, pattern=pattern)
    return _CACHE[key]


def _host_combine(results, norm_means):
    """results: per-core dicts with 'out' [128,2]; norm_means: [B,2] f32."""
    losses = []
    for b in range(B):
        o = results[b]["out"].astype(np.float64)
        t1 = o[:, 0].sum() / N + norm_means[b, 0]
        t2 = o[:, 1].sum() / M + norm_means[b, 1]
        losses.append(t1 + t2)
    return np.float32(np.mean(losses))


def kernel(f, f_):
    from concourse.bass_utils import run_bass_kernel_spmd

    assert f.shape == (B, N, C) and f_.shape == (B, M, C)
    nc = _get_program(num_devices=B)

    in_maps = []
    norm_means = np.zeros((B, 2), np.float64)
    for b in range(B):
        fb = np.asarray(f[b], np.float64)
        gb = np.asarray(f_[b], np.float64)
        in_maps.append(_prep_batch(np.asarray(f[b]), np.asarray(f_[b])))
        norm_means[b, 0] = (fb * fb).sum() / N
        norm_means[b, 1] = (gb * gb).sum() / M
    last_err = None
    for _ in range(4):
        try:
            res = run_bass_kernel_spmd(nc, in_maps, core_ids=list(range(B)))
            return _host_combine(res.results, norm_means)
        except Exception as e:  # transient device-unrecoverable flakes
            last_err = e
    raise last_err



# revision 8
# speedup vs baseline: 3.7115x; 3.7115x over previous
"""Chamfer loss kernel for 8 Trainium2 NeuronCores.

Problem: f, f_ of shape [8, 4096, 3] fp32; loss = mean_b [ mean_n min_m D + mean_m min_n D ]
where D is the [4096, 4096] squared-distance matrix per batch.

Sharding: batch-parallel, one batch per core (8 cores).

Single-pass design (v2): the FULL distance D[n,m] = ||f_n||^2 - 2 f.g + ||g_m||^2
is produced by ONE augmented bf16 matmul (K=18 contraction: 12 rows of hi/lo
product splits, 3 rows carrying a 3-way split of ||g_m||^2, 3 rows carrying a
3-way split of ||f_n||^2).  Both min directions are then extracted from the
same PSUM tiles, which halves tensor-engine work AND PSUM egress vs computing
D and D^T separately:

  row-mins (min over m): ScalarE copies each [128,2048] PSUM tile to SBUF
    bf16; DVE folds pairs with tensor_tensor(min) (2x/4x perf modes) down to
    256 cols per block, stored in a row accumulator; a segmented fold tree at
    the end produces per-block row-mins and their per-partition sums.
  col-mins (min over n): DVE keeps a running bf16 min accumulator
    R[128, 4096] = min over blocks of the copied tiles (one 4x TT per tile).
    At the end the PE array transposes R in 32 [128,128] chunks (bf16
    transpose into PSUM) and DVE folds across the old partition axis.

Out is [128, 2] per-partition partial sums (col 0: row-min sums, col 1:
col-min sums); the host sums partitions and averages over batches.  No host
norm correction is needed -- the matmul emits the complete distance.

Per-core engine budget (cost model): Act 64 copies ~121us (critical), DVE
~100us, PE 256 bf16 matmuls 55-109us (p-state dependent).  The optional
`pattern` knob marks blocks as 'A' (DVE reduces/accumulates straight from
PSUM fp32, no ScalarE copy) to rebalance Act vs DVE load.
"""

import os
import sys

import numpy as np

for _p in ("/opt/trn_rl_repo",):
    if _p not in sys.path and os.path.isdir(_p):
        sys.path.append(_p)

import ml_dtypes  # noqa: E402

BF16 = ml_dtypes.bfloat16

B, N, M, C = 8, 4096, 4096, 3
NBLK = 128          # rows per n-block (PSUM partition dim)
MTILE = 2048        # columns per PSUM tile (fp32 -> 4 banks)
MMN = 512           # matmul free dim (one PSUM bank of fp32)
K = 18              # augmented contraction dim


# ----------------------------------------------------------------- host prep
def _bf16_split(x):
    """x (f32/f64) -> (hi, lo) bf16 arrays with hi+lo ~ x (16-bit mantissa)."""
    hi = x.astype(BF16)
    lo = (x.astype(np.float64) - hi.astype(np.float64)).astype(BF16)
    return hi, lo


def _split3(v):
    """v (f64 [n]) -> three bf16 rows summing to ~v (24-bit mantissa)."""
    a = v.astype(BF16)
    b = (v - a.astype(np.float64)).astype(BF16)
    c = (v - a.astype(np.float64) - b.astype(np.float64)).astype(BF16)
    return a, b, c


def _prep_batch(f, g):
    """Build the augmented [K, 4096] bf16 operands for one batch.

    W(f): stationary: rows [yh,yh,yl,yl (3 each), 1,1,1, m1,m2,m3]
    S(g): moving:     rows [xh,xl,xh,xl (3 each), n1,n2,n3, 1,1,1]
    with y = -2 f, x = g, m* = 3-way split of ||f_n||^2, n* = split of
    ||g_m||^2, so W(f).T @ S(g) = ||f_n||^2 - 2 f.g + ||g_m||^2 = D[n,m].
    """
    fd = f.astype(np.float64)
    gd = g.astype(np.float64)

    w = np.zeros((K, f.shape[0]), dtype=BF16)
    yh, yl = _bf16_split(-2.0 * fd)
    w[0:3] = yh.T
    w[3:6] = yh.T
    w[6:9] = yl.T
    w[9:12] = yl.T
    w[12:15] = np.ones((3, f.shape[0]), dtype=BF16)
    m1, m2, m3 = _split3((fd * fd).sum(axis=1))
    w[15], w[16], w[17] = m1, m2, m3

    s = np.zeros((K, g.shape[0]), dtype=BF16)
    xh, xl = _bf16_split(gd)
    s[0:3] = xh.T
    s[3:6] = xl.T
    s[6:9] = xh.T
    s[9:12] = xl.T
    n1, n2, n3 = _split3((gd * gd).sum(axis=1))
    s[12], s[13], s[14] = n1, n2, n3
    s[15:18] = np.ones((3, g.shape[0]), dtype=BF16)

    return {
        "wf": np.ascontiguousarray(w),
        "sg": np.ascontiguousarray(s),
    }


# ------------------------------------------------------------- device program
def build_program(num_devices, n_points=N, m_points=M, repeat=1, hw_repeat=1,
                  pattern="B"):
    """Build the Bass program. Returns nc.

    pattern: per-block schedule, cycled to cover all blocks. 'B' = ScalarE
      copies both PSUM tiles to SBUF bf16; DVE runs fast bf16 TT-min folds.
      'A' = DVE reduces/accumulates straight from PSUM fp32 (no ScalarE) --
      use a sparse sprinkle of 'A' to offload the ScalarE critical path.
    """
    import concourse.bass as bass  # noqa: F401
    import concourse.mybir as mybir
    from concourse import bacc, tile

    f32 = mybir.dt.float32
    bf16 = mybir.dt.bfloat16
    AL = mybir.AluOpType
    AX = mybir.AxisListType.X

    nb = n_points // NBLK           # n-blocks
    nt = m_points // MTILE          # psum tiles per block
    assert nt == 2

    nc = bacc.Bacc("TRN2", target_bir_lowering=False, debug=False,
                   num_devices=num_devices)

    wf = nc.dram_tensor("wf", [K, n_points], bf16, kind="ExternalInput")
    sg = nc.dram_tensor("sg", [K, m_points], bf16, kind="ExternalInput")
    out = nc.dram_tensor("out", [128, 2], f32, kind="ExternalOutput")

    plan = (pattern * nb)[:nb]
    na = plan.count("A")

    with tile.TileContext(nc) as tc:
        with (
            tc.tile_pool(name="inp", bufs=1) as inp,
            tc.tile_pool(name="psum", bufs=2, space="PSUM") as psum,
            tc.tile_pool(name="cpp", bufs=6) as cpp,
            tc.tile_pool(name="tp", bufs=2) as tp,
            tc.tile_pool(name="up", bufs=2) as up,
            tc.tile_pool(name="vp", bufs=2) as vp,
            tc.tile_pool(name="r0p", bufs=2) as r0p,
            tc.tile_pool(name="r1p", bufs=2) as r1p,
            tc.tile_pool(name="rap", bufs=2) as rap,
            tc.tile_pool(name="fp", bufs=2) as fp,
            tc.tile_pool(name="outp", bufs=2) as outp,
        ):
            wf_t = inp.tile([K, n_points], bf16, tag="wf")
            sg_t = inp.tile([K, m_points], bf16, tag="sg")
            nc.sync.dma_start(wf_t[:], wf.ap())
            nc.sync.dma_start(sg_t[:], sg.ap())

            def body(_iv=None):
                out_t = outp.tile([128, 2], f32, tag="out")
                # row accumulator: 256 folded cols per B block, 2 direct
                # reduce cols per A block
                ra = None
                if na < nb:
                    ra = rap.tile([128, (nb - na) * 256], bf16, tag="ra",
                                  name="ra")
                minA = None
                if na:
                    minA = fp.tile([128, 2 * na], f32, tag="minA",
                                   name="minA")
                state = {"R0": None, "R1": None}
                rpools = (r0p, r1p)
                pending = []
                bcol = 0
                acol = 0

                def make_b(i, cps, col):
                    def emit():
                        # col-min chains (one TT per tile, 4x mode)
                        for t in range(nt):
                            key = f"R{t}"
                            rn = rpools[t].tile([128, MTILE], bf16, tag=key)
                            if state[key] is None:
                                nc.vector.tensor_copy(rn[:], cps[t][:])
                            else:
                                nc.vector.tensor_tensor(
                                    out=rn[:], in0=cps[t][:],
                                    in1=state[key][:], op=AL.min)
                            state[key] = rn
                        # row-min fold 4096 -> 256
                        t_ = tp.tile([128, 2048], bf16, tag="t")
                        nc.vector.tensor_tensor(
                            out=t_[:], in0=cps[0][:], in1=cps[1][:], op=AL.min)
                        u_ = up.tile([128, 1024], bf16, tag="u")
                        nc.vector.tensor_tensor(
                            out=u_[:], in0=t_[:, 0:1024], in1=t_[:, 1024:2048],
                            op=AL.min)
                        v_ = vp.tile([128, 512], bf16, tag="v")
                        nc.vector.tensor_tensor(
                            out=v_[:], in0=u_[:, 0:512], in1=u_[:, 512:1024],
                            op=AL.min)
                        nc.vector.tensor_tensor(
                            out=ra[:, 256 * col:256 * (col + 1)],
                            in0=v_[:, 0:256], in1=v_[:, 256:512], op=AL.min)
                    return emit

                def make_a(i, pts, col):
                    def emit():
                        for t in range(nt):
                            key = f"R{t}"
                            rn = rpools[t].tile([128, MTILE], bf16, tag=key)
                            if state[key] is None:
                                nc.vector.tensor_copy(rn[:], pts[t][:])
                            else:
                                nc.vector.tensor_tensor(
                                    out=rn[:], in0=pts[t][:],
                                    in1=state[key][:], op=AL.min)
                            state[key] = rn
                            nc.vector.tensor_reduce(
                                out=minA[:, 2 * col + t:2 * col + t + 1],
                                in_=pts[t][:], axis=AX, op=AL.min)
                    return emit

                for i in range(nb):
                    typ = plan[i]
                    lhsT = wf_t[0:K, NBLK * i:NBLK * (i + 1)]
                    cps = []
                    pts = []
                    for t in range(nt):
                        pt = psum.tile([128, MTILE], f32, tag="ps")
                        for h in range(MTILE // MMN):
                            m0 = MTILE * t + MMN * h
                            nc.tensor.matmul(
                                pt[:, MMN * h:MMN * (h + 1)],
                                lhsT,
                                sg_t[0:K, m0:m0 + MMN],
                                start=True, stop=True,
                            )
                        pts.append(pt)
                        if typ == "B":
                            cp = cpp.tile([128, MTILE], bf16, tag=f"cp{t}")
                            nc.scalar.copy(cp[:], pt[:])
                            cps.append(cp)
                    # one-block delay so the DVE FIFO never head-blocks on
                    # ScalarE copies still in flight
                    if len(pending) > 1:
                        pending.pop(0)()
                    if typ == "B":
                        pending.append(make_b(i, cps, bcol))
                        bcol += 1
                    else:
                        pending.append(make_a(i, pts, acol))
                        acol += 1
                for fn in pending:
                    fn()

                # ---- row-min epilogue ----
                parts = []
                if ra is not None:
                    # segmented fold tree [128,(nbB,256)] -> [128,nbB]
                    cur, width = ra, 256
                    nbb = nb - na
                    while width > 1:
                        half = width // 2
                        nxt = fp.tile([128, nbb * half], bf16,
                                      tag=f"rf{width}")
                        a3 = cur[:].rearrange("p (i q) -> p i q", q=width)
                        o3 = nxt[:].rearrange("p (i q) -> p i q", q=half)
                        nc.vector.tensor_tensor(
                            out=o3, in0=a3[:, :, 0:half],
                            in1=a3[:, :, half:width], op=AL.min)
                        cur, width = nxt, half
                    pb = fp.tile([128, 1], f32, tag="pb")
                    nc.vector.tensor_reduce(
                        out=pb[:], in_=cur[:], axis=AX, op=AL.add)
                    parts.append(pb)
                if na:
                    # pair the two tile-mins of each A block, then sum
                    mv = minA[:, 0:2 * na].rearrange("p (i q) -> p i q", q=2)
                    sc = fp.tile([128, na], f32, tag="scA")
                    nc.vector.tensor_reduce(
                        out=sc[:], in_=mv, axis=AX, op=AL.min)
                    pa = fp.tile([128, 1], f32, tag="pa")
                    nc.vector.tensor_reduce(
                        out=pa[:], in_=sc[:], axis=AX, op=AL.add)
                    parts.append(pa)
                if len(parts) == 2:
                    nc.vector.tensor_tensor(
                        out=out_t[:, 0:1], in0=parts[0][:], in1=parts[1][:],
                        op=AL.add)
                else:
                    nc.vector.tensor_copy(out_t[:, 0:1], parts[0][:])

                # ---- col-min epilogue: DMA-crossbar transpose of R into
                # [q, chunk, p] layout, then fold the old partition axis ----
                nchunk = m_points // 128     # 32
                per_r = MTILE // 128         # 16
                tb = fp.tile([128, nchunk * 128], bf16, tag="tb")
                t3 = tb[:].rearrange("p (c q) -> p c q", q=128)
                nc.sync.dma_start_transpose(t3[:, 0:per_r, :], state["R0"][:])
                nc.sync.dma_start_transpose(
                    t3[:, per_r:2 * per_r, :], state["R1"][:])
                cur, width = tb, 128
                while width > 1:
                    half = width // 2
                    nxt = fp.tile([128, nchunk * half], bf16,
                                  tag=f"cf{width}")
                    a3 = cur[:].rearrange("p (i q) -> p i q", q=width)
                    o3 = nxt[:].rearrange("p (i q) -> p i q", q=half)
                    nc.vector.tensor_tensor(
                        out=o3, in0=a3[:, :, 0:half], in1=a3[:, :, half:width],
                        op=AL.min)
                    cur, width = nxt, half
                nc.vector.tensor_reduce(
                    out=out_t[:, 1:2], in_=cur[:], axis=AX, op=AL.add)

                nc.sync.dma_start(out.ap(), out_t[:])

            if hw_repeat > 1:
                with tc.For_i(0, hw_repeat, 1) as iv:
                    for _ in range(repeat):
                        body(iv)
            else:
                for _ in range(repeat):
                    body()

    nc.compile()
    return nc


# ----------------------------------------------------------------- entrypoint
_CACHE = {}


def _get_program(num_devices=8, repeat=1, hw_repeat=1, pattern="B"):
    key = (num_devices, repeat, hw_repeat, pattern)
    if key not in _CACHE:
        _CACHE[key] = build_program(num_devices, repeat=repeat,
                                    hw_repeat=hw_repeat, pattern=pattern)
    return _CACHE[key]


def _host_combine(results):
    """results: per-core dicts with 'out' [128,2] partial sums."""
    losses = []
    for b in range(B):
        o = results[b]["out"].astype(np.float64)
        losses.append(o[:, 0].sum() / N + o[:, 1].sum() / M)
    return np.float32(np.mean(losses))


def kernel(f, f_):
    from concourse.bass_utils import run_bass_kernel_spmd

    assert f.shape == (B, N, C) and f_.shape == (B, M, C)
    nc = _get_program(num_devices=B)

    in_maps = []
    for b in range(B):
        in_maps.append(_prep_batch(np.asarray(f[b]), np.asarray(f_[b])))
    last_err = None
    for _ in range(4):
        try:
            res = run_bass_kernel_spmd(nc, in_maps, core_ids=list(range(B)))
            return _host_combine(res.results)
        except Exception as e:  # transient device-unrecoverable flakes
            last_err = e
    raise last_err
